# revision 33
# baseline (speedup 1.0000x reference)
"""BVH ray/leaf-AABB intersection kernel for 8 Trainium2 NeuronCores.

Strategy (pure data parallelism): shard the 262144 rays across 8 cores
(32768 rays each), replicate the 64-leaf AABB table on every core.

Per-core pipeline:
  Phase A (DVE/ACT): ray setup — origin o, unit direction v (Newton-refined
    rsqrt), inv = 1/v (hardware iterative divide), coefficients
    ip=max(inv,0), im=min(inv,0), q=o*inv, each split into 3 bf16 parts
    (exact 24-bit decomposition).
  Phase B (PE): per 128-ray tile one K=45 bf16 matmul computes, for all 64
    leaves and 3 axes, the sign-selected slab distances
      tmin_c = ip_c*LM_c + im_c*LX_c - q_c
      tmax_c = im_c*LM_c + ip_c*LX_c - q_c
    via the 6-product bf16 expansion per product pair (error ~1 fp32 ulp,
    validated to preserve every hit/argmin decision on the target data).
    Coefficients are transposed to lhsT layout with PE transposes; the
    PSUM->SBUF copies ride on the otherwise-idle ScalarE.
  Phase C (DVE/GPSIMD/ACT): free-dim reduces give tnear/tfar; slab hit
    test on GPSIMD; argmin-select with a BIG-fill built on ScalarE; t2 via
    one-hot max; entry/exit points assembled on DVE.

Ray layout per core: ray = p*256 + t (p = SBUF partition, t = tile index),
so every DMA is contiguous per partition.
"""

import numpy as np
import ml_dtypes

import concourse.bass as bass
import concourse.bacc as bacc
import concourse.mybir as mybir
import concourse.tile as tile
from concourse.bass_utils import run_bass_kernel_spmd
from concourse.masks import make_identity

F32 = mybir.dt.float32
BF16 = mybir.dt.bfloat16
ALU = mybir.AluOpType
AX = mybir.AxisListType
AF = mybir.ActivationFunctionType

N_RAYS = 262144
N_CORES = 8
NC_RAYS = N_RAYS // N_CORES      # 32768
P = 128                          # partitions
T = NC_RAYS // P                 # 256 tiles (rays per partition)
L = 64                           # leaves
NMM = 2 * 3 * L                  # 384 matmul output columns (side, comp, leaf)
GT = 4                           # tiles per PSUM group (4 banks)
NG = T // GT                     # 64 groups
NCH = T // 2                     # 128 transpose chunks (2 tiles each)
K = 45                           # matmul contraction rows
BIG = np.float32(1.0e30)
BIGH = float(np.float32(5.0e29))
MINPOS = float(np.float32(1.1754943508222875e-38))  # min normal fp32
USE_DMA_TRANSPOSE = True
USE_TTR = False

# per-component 15-row block: 6 ip-pair rows, 6 im-pair rows, 3 q rows.
# coefficient part index (1-based) for the 6 product rows (a1b1, a1b2,
# a2b1, a1b3, a2b2, a3b1):
_APART = [1, 1, 2, 1, 2, 3]
_BPART = [1, 2, 1, 3, 2, 1]


def _build_nc() -> bass.Bass:
    nc = bacc.Bacc()
    xc = nc.dram_tensor("xc", [NC_RAYS, 6], F32, kind="ExternalInput")
    rhsf = nc.dram_tensor("rhsf", [P, NMM], BF16, kind="ExternalInput")
    out6 = nc.dram_tensor("out6", [NC_RAYS, 6], F32, kind="ExternalOutput")
    t1o = nc.dram_tensor("t1o", [NC_RAYS], F32, kind="ExternalOutput")
    t2o = nc.dram_tensor("t2o", [NC_RAYS], F32, kind="ExternalOutput")
    msk = nc.dram_tensor("msk", [NC_RAYS], F32, kind="ExternalOutput")

    with tile.TileContext(nc) as tc:
        _body(tc, xc, rhsf, out6, t1o, t2o, msk)
    nc.compile()
    return nc


def _body(tc, xc, rhsf, out6, t1o, t2o, msk):
    nc = tc.nc
    v = nc.vector
    g = nc.gpsimd

    with (
        tc.tile_pool(name="persist", bufs=1) as persist,
        tc.tile_pool(name="work", bufs=4) as work,
    ):
        # ---------------- persistent SBUF buffers ----------------
        X = persist.tile([P, T, 6], F32)       # rays (o | endpoint)
        VU = persist.tile([P, T, 3], F32)      # unnormalized direction
        SQ = persist.tile([P, T, 3], F32)
        N2 = persist.tile([P, T], F32)
        R = persist.tile([P, T], F32)          # 1/norm
        A1 = persist.tile([P, T], F32)
        A2 = persist.tile([P, T], F32)
        A3 = persist.tile([P, T], F32)
        R3 = persist.tile([P, T, 3], F32)
        VEC = persist.tile([P, T, 3], F32)     # unit direction
        INV = persist.tile([P, T, 3], F32)     # 1/vec
        QQ = persist.tile([P, T, 3], F32)      # o*inv
        IP = persist.tile([P, T, 3], F32)
        IM = persist.tile([P, T, 3], F32)
        RS1 = persist.tile([P, T, 3], F32)     # split residuals
        RS2 = persist.tile([P, T, 3], F32)
        PARTS = {}
        for nm in ("ip", "im", "q"):
            for j in (1, 2, 3):
                PARTS[(nm, j)] = persist.tile([P, T, 3], BF16,
                                              name=f"pt_{nm}{j}", tag=f"pt_{nm}{j}")
        C64 = persist.tile([P, T, 64], BF16)   # padded coeff slots for transpose
        LHS = persist.tile([P, NCH, P], BF16)  # transposed coeffs (lhsT source)
        RHS = persist.tile([P, NMM], BF16)     # leaf-table moving operand
        OUT6 = persist.tile([P, T, 6], F32)
        T1B = persist.tile([P, T], F32)
        T2B = persist.tile([P, T], F32)
        MKB = persist.tile([P, T], F32)
        M1B = persist.tile([P, T], F32)

        # ---------------- input DMA + constants ----------------
        nc.sync.dma_start(out=X[:], in_=xc[:].rearrange("(p t) c -> p t c", p=P))
        nc.sync.dma_start(out=RHS[:], in_=rhsf[:])
        g.memset(C64[:], 0.0)

        # ---------------- Phase A: per-ray setup ----------------
        O = X[:, :, 0:3]
        v.tensor_tensor(out=VU[:], in0=X[:, :, 3:6], in1=O, op=ALU.subtract)
        nc.scalar.activation(out=SQ[:], in_=VU[:], func=AF.Square)
        v.tensor_reduce(out=N2[:], in_=SQ[:], axis=AX.X, op=ALU.add)
        nc.scalar.activation(out=A1[:], in_=N2[:], func=AF.Sqrt)
        v.reciprocal(out=R[:], in_=A1[:])
        # three Newton steps: r <- r*(1.5 - 0.5*n2*r*r)  (ACT sqrt is coarse)
        for _ in range(3):
            v.tensor_tensor(out=A1[:], in0=R[:], in1=R[:], op=ALU.mult)
            v.tensor_tensor(out=A2[:], in0=A1[:], in1=N2[:], op=ALU.mult)
            v.tensor_scalar(out=A3[:], in0=A2[:], scalar1=-0.5, scalar2=1.5,
                            op0=ALU.mult, op1=ALU.add)
            v.tensor_tensor(out=R[:], in0=R[:], in1=A3[:], op=ALU.mult)
        for c in range(3):
            nc.scalar.copy(out=R3[:, :, c], in_=R[:])
        v.tensor_tensor(out=VEC[:], in0=VU[:], in1=R3[:], op=ALU.mult)
        v.reciprocal(out=INV[:], in_=VEC[:])
        v.tensor_tensor(out=QQ[:], in0=O, in1=INV[:], op=ALU.mult)
        v.tensor_single_scalar(out=IP[:], in_=INV[:], scalar=0.0, op=ALU.max)
        v.tensor_single_scalar(out=IM[:], in_=INV[:], scalar=0.0, op=ALU.min)

        # exact 3-part bf16 splits of ip, im, q
        for nm, SRC in (("ip", IP), ("im", IM), ("q", QQ)):
            p1, p2, p3 = PARTS[(nm, 1)], PARTS[(nm, 2)], PARTS[(nm, 3)]
            nc.scalar.copy(out=p1[:], in_=SRC[:])
            v.tensor_tensor(out=RS1[:], in0=SRC[:], in1=p1[:], op=ALU.subtract)
            nc.scalar.copy(out=p2[:], in_=RS1[:])
            v.tensor_tensor(out=RS2[:], in0=RS1[:], in1=p2[:], op=ALU.subtract)
            nc.scalar.copy(out=p3[:], in_=RS2[:])

        # distribute parts into the 45 used coefficient slots (on GPSIMD)
        CS = C64[:, :, 0:45].rearrange("p t (c s) -> p t c s", s=15)
        for nm, base in (("ip", 0), ("im", 6)):
            for row, ap in enumerate(_APART):
                g.tensor_copy(out=CS[:, :, :, base + row], in_=PARTS[(nm, ap)][:])
        for j in (1, 2, 3):
            g.tensor_copy(out=CS[:, :, :, 11 + j], in_=PARTS[("q", j)][:])

        # ---------------- Phase B-a: transpose coeffs to lhsT layout ----------
        if USE_DMA_TRANSPOSE:
            # xbar DMA transpose (bf16): [128 rays, 128 coeff-slots] -> lhsT rows
            for ch in range(NCH):
                nc.sync.dma_start_transpose(
                    LHS[:, ch, :],
                    C64[:, 2 * ch:2 * (ch + 1), :].rearrange("p t k -> p (t k)"),
                )
        else:
            ID = persist.tile([P, P], BF16)
            make_identity(nc, ID[:])
            with tc.tile_pool(name="tpp", bufs=2, space="PSUM") as tpp:
                for ch in range(NCH):
                    tp = tpp.tile([P, P], BF16)
                    nc.tensor.transpose(
                        tp[:],
                        C64[:, 2 * ch:2 * (ch + 1), :].rearrange("p t k -> p (t k)"),
                        ID[:],
                    )
                    nc.scalar.copy(out=LHS[:, ch, :], in_=tp[:])

        # ---------------- Phase B-b + C: matmuls and selection ----------------
        with tc.tile_pool(name="mmp", bufs=2, space="PSUM") as mmp:
            for grp in range(NG):
                MM = mmp.tile([P, GT, 512], F32)
                for j in range(GT):
                    t = GT * grp + j
                    ch, tl = t // 2, t % 2
                    base = 64 * tl
                    nc.tensor.matmul(
                        MM[:, j, 0:NMM],
                        lhsT=LHS[base:base + K, ch, :],
                        rhs=RHS[base:base + K, :],
                        start=True, stop=True,
                        tile_position=(base, 0),
                    )

                g4 = slice(GT * grp, GT * (grp + 1))
                near = MM[:, :, 0:192].rearrange("p t (c l) -> p t l c", c=3)
                far = MM[:, :, 192:384].rearrange("p t (c l) -> p t l c", c=3)
                TN = work.tile([P, GT, L], F32)
                TF = work.tile([P, GT, L], F32)
                HIT = work.tile([P, GT, L], F32)
                U = work.tile([P, GT, L], F32)
                V = work.tile([P, GT, L], F32)
                PICK = work.tile([P, GT, L], F32)
                W = work.tile([P, GT, L], F32)
                T2R = work.tile([P, GT], F32)
                TMP = work.tile([P, GT, 3], F32)
                TMP2 = work.tile([P, GT, 3], F32)

                v.tensor_reduce(out=TN[:], in_=near, axis=AX.X, op=ALU.max)
                v.tensor_reduce(out=TF[:], in_=far, axis=AX.X, op=ALU.min)
                # hit = (max(tnear, MINPOS) <= tfar)  [== (tfar>=max(tn,0)) & (tfar>0)]
                v.scalar_tensor_tensor(out=HIT[:], in0=TN[:], scalar=MINPOS,
                                       in1=TF[:], op0=ALU.max, op1=ALU.is_le)
                # V = hit ? tnear : BIG   (U = BIG*(1-hit) on ScalarE; BIG absorbs TN)
                nc.scalar.activation(out=U[:], in_=HIT[:], func=AF.Copy,
                                     scale=-float(BIG), bias=float(BIG))
                # V = TN + U, then argmin reduce
                if USE_TTR:
                    for j in range(GT):
                        t = GT * grp + j
                        v.tensor_tensor_reduce(
                            out=V[:, j, :], in0=TN[:, j, :], in1=U[:, j, :],
                            scale=1.0, scalar=float(BIG), op0=ALU.add, op1=ALU.min,
                            accum_out=M1B[:, t:t + 1])
                else:
                    v.tensor_tensor(out=V[:], in0=TN[:], in1=U[:], op=ALU.add)
                    v.tensor_reduce(out=M1B[:, g4], in_=V[:], axis=AX.X, op=ALU.min)
                M1 = M1B[:, g4]
                MK = MKB[:, g4]
                v.tensor_single_scalar(out=MK, in_=M1, scalar=BIGH, op=ALU.is_lt)
                T1 = T1B[:, g4]
                v.tensor_tensor(out=T1, in0=M1, in1=MK, op=ALU.mult)
                v.tensor_tensor(out=PICK[:], in0=V[:],
                                in1=M1.to_broadcast([P, GT, L]), op=ALU.is_equal)
                # W = PICK * TF, then one-hot max
                if USE_TTR:
                    for j in range(GT):
                        v.tensor_tensor_reduce(
                            out=W[:, j, :], in0=PICK[:, j, :], in1=TF[:, j, :],
                            scale=1.0, scalar=0.0, op0=ALU.mult, op1=ALU.max,
                            accum_out=T2R[:, j:j + 1])
                else:
                    v.tensor_tensor(out=W[:], in0=PICK[:], in1=TF[:], op=ALU.mult)
                    v.tensor_reduce(out=T2R[:], in_=W[:], axis=AX.X, op=ALU.max)
                T2 = T2B[:, g4]
                v.tensor_tensor(out=T2, in0=T2R[:], in1=MK, op=ALU.mult)
                # points: o + vec*t
                v.tensor_tensor(out=TMP[:], in0=VEC[:, g4, :],
                                in1=T1.to_broadcast([P, GT, 3]), op=ALU.mult)
                v.tensor_tensor(out=OUT6[:, g4, 0:3], in0=TMP[:],
                                in1=X[:, g4, 0:3], op=ALU.add)
                v.tensor_tensor(out=TMP2[:], in0=VEC[:, g4, :],
                                in1=T2.to_broadcast([P, GT, 3]), op=ALU.mult)
                v.tensor_tensor(out=OUT6[:, g4, 3:6], in0=TMP2[:],
                                in1=X[:, g4, 0:3], op=ALU.add)

        # ---------------- Phase D: output DMA ----------------
        nc.sync.dma_start(out=out6[:].rearrange("(p t) c -> p t c", p=P), in_=OUT6[:])
        nc.sync.dma_start(out=t1o[:].rearrange("(p t) -> p t", p=P), in_=T1B[:])
        nc.sync.dma_start(out=t2o[:].rearrange("(p t) -> p t", p=P), in_=T2B[:])
        nc.sync.dma_start(out=msk[:].rearrange("(p t) -> p t", p=P), in_=MKB[:])


def _split3(a):
    bf = ml_dtypes.bfloat16
    f32 = np.float32
    a = a.astype(f32)
    a1 = a.astype(bf).astype(f32)
    r = (a - a1).astype(f32)
    a2 = r.astype(bf).astype(f32)
    a3 = (r - a2).astype(f32).astype(bf).astype(f32)
    return a1, a2, a3


def _build_rhs(leaf_min: np.ndarray, leaf_max: np.ndarray) -> np.ndarray:
    """[128, 384] bf16 moving operand: replicated at partition bases 0 and 64,
    45 rows each: per component, 6 ip-pair rows, 6 im-pair rows, 3 q rows."""
    lmp = _split3(leaf_min)   # tuple of 3 fp32 [64,3]
    lxp = _split3(leaf_max)
    r = np.zeros((P, NMM), np.float32)
    for b in (0, 64):
        for c in range(3):
            near = slice(c * L, (c + 1) * L)
            far = slice(192 + c * L, 192 + (c + 1) * L)
            rb = b + 15 * c
            for row, bp in enumerate(_BPART):
                # ip rows pair with LM on the near side, LX on the far side
                r[rb + row, near] = lmp[bp - 1][:, c]
                r[rb + row, far] = lxp[bp - 1][:, c]
                # im rows pair the other way
                r[rb + 6 + row, near] = lxp[bp - 1][:, c]
                r[rb + 6 + row, far] = lmp[bp - 1][:, c]
            for j in range(3):
                r[rb + 12 + j, near] = -1.0
                r[rb + 12 + j, far] = -1.0
    return r.astype(ml_dtypes.bfloat16)


_NC_CACHE = None


def _get_nc():
    global _NC_CACHE
    if _NC_CACHE is None:
        _NC_CACHE = _build_nc()
    return _NC_CACHE


def kernel(x: np.ndarray, leaf_min: np.ndarray, leaf_max: np.ndarray, _trace=False):
    nc = _get_nc()
    x = np.ascontiguousarray(x, dtype=np.float32)
    rhs = _build_rhs(leaf_min, leaf_max)
    in_maps = [
        {"xc": np.ascontiguousarray(x[i * NC_RAYS:(i + 1) * NC_RAYS]), "rhsf": rhs}
        for i in range(N_CORES)
    ]
    res = run_bass_kernel_spmd(nc, in_maps, core_ids=list(range(N_CORES)),
                               trace=_trace)
    outs = res.results
    out = np.concatenate([r["out6"] for r in outs], axis=0)
    t1 = np.concatenate([r["t1o"] for r in outs], axis=0)[:, None]
    t2 = np.concatenate([r["t2o"] for r in outs], axis=0)[:, None]
    mask = np.concatenate([r["msk"] for r in outs], axis=0) > 0.5
    if _trace:
        kernel._last_result = res
    return out, mask, t1, t2
